# revision 1
# baseline (speedup 1.0000x reference)
"""Trainium2 Bass kernel for nn_LogisticRegressionRBF.

reference:
    sq[i,j] = ||x_i||^2 + ||c_j||^2 - 2 x_i.c_j     (K=65536 obs, N=4096 basis)
    out     = sigmoid(exp(-sq) @ w.T + b)           [K, 1]

Strategy (data-parallel over obs across 8 cores, 8192 obs/core):
  * Augmented contraction folds the full distance into ONE matmul:
       stationary cte[:, j] = [2*c_j (64) ; -1 ; -||c_j||^2]   (66 rows)
       moving    xte[:, i] = [x_i  (64) ; ||x_i||^2 ; 1]
       G[j, i]  = cte.T @ xte = -sq[i, j]           (PSUM, basis on partitions)
  * ACT exp on [128, 2048]/[128, 1536] PSUM blocks (pure throughput, 1 elem/
    lane/cyc) -> phi in SBUF.
  * PE matvec with w tile [128, 1] reduces over basis partitions,
    accumulating z[1, 512] in PSUM across the 32 basis tiles.
  * sigmoid via tanh (same ACT table set as exp -> no table reload):
       sigmoid(z + b) = 0.5 + 0.5 * tanh(0.5*z + 0.5*b)
PSUM budget: gA 4 banks + gB 3 banks + z 1 bank = 8.
"""

import sys

if "/opt/trn_rl_repo" not in sys.path:
    sys.path.insert(0, "/opt/trn_rl_repo")

import numpy as np

K_FULL, N_BASIS, M_FEAT = 65536, 4096, 64
N_CORES = 8
K_SHARD = K_FULL // N_CORES  # 8192
KE = M_FEAT + 2  # 66 augmented contraction rows
OBS_SLICE = 512  # matmul moving free dim (one PSUM bank of fp32)
BAS_TILE = 128  # basis per stationary tile / psum partitions
# basis tiles per ACT block: alternate 4-tile (2048 col) / 3-tile (1536 col)
BLOCK_SIZES = [4, 3, 4, 3, 4, 3, 4, 3, 4]  # sums to 32 = N_BASIS / BAS_TILE

_prog_cache: dict = {}


def _build_program():
    import concourse.bacc as bacc
    import concourse.bass as bass
    import concourse.mybir as mybir
    import concourse.tile as tile

    f32 = mybir.dt.float32
    nc = bacc.Bacc("TRN2", target_bir_lowering=False, debug=False,
                   num_devices=N_CORES)

    xte_d = nc.dram_tensor("xte", [KE, K_SHARD], f32, kind="ExternalInput")
    cte_d = nc.dram_tensor("cte", [KE, N_BASIS], f32, kind="ExternalInput")
    wsb_d = nc.dram_tensor("wsb", [BAS_TILE, N_BASIS // BAS_TILE], f32,
                           kind="ExternalInput")
    hb_d = nc.dram_tensor("hb", [1, 1], f32, kind="ExternalInput")
    out_d = nc.dram_tensor("out", [1, K_SHARD], f32, kind="ExternalOutput")

    n_slices = K_SHARD // OBS_SLICE  # 16
    n_btiles = N_BASIS // BAS_TILE  # 32
    X_CHUNK = 2048  # xte DMA chunk (cols)
    C_CHUNK = 512  # cte DMA chunk (cols)

    with tile.TileContext(nc) as tc:
        with (
            tc.tile_pool(name="const", bufs=1) as const,
            tc.tile_pool(name="xch", bufs=K_SHARD // X_CHUNK) as xpool,
            tc.tile_pool(name="cch", bufs=N_BASIS // C_CHUNK) as cpool,
            tc.tile_pool(name="phi", bufs=3) as ppool,
            tc.tile_pool(name="tsb", bufs=2) as tpool,
            tc.tile_pool(name="gA", bufs=1, space="PSUM") as gApool,
            tc.tile_pool(name="gB", bufs=1, space="PSUM") as gBpool,
            tc.tile_pool(name="zps", bufs=1, space="PSUM") as zpool,
        ):
            wsb = const.tile([BAS_TILE, n_btiles], f32)
            nc.sync.dma_start(out=wsb[:], in_=wsb_d.ap())
            hb = const.tile([1, 1], f32)
            nc.sync.dma_start(out=hb[:], in_=hb_d.ap())

            ctiles = []
            for j in range(N_BASIS // C_CHUNK):
                ct = cpool.tile([KE, C_CHUNK], f32)
                nc.sync.dma_start(
                    out=ct[:], in_=cte_d.ap()[:, j * C_CHUNK:(j + 1) * C_CHUNK]
                )
                ctiles.append(ct)

            xtiles = []
            for j in range(K_SHARD // X_CHUNK):
                xt = xpool.tile([KE, X_CHUNK], f32)
                nc.sync.dma_start(
                    out=xt[:], in_=xte_d.ap()[:, j * X_CHUNK:(j + 1) * X_CHUNK]
                )
                xtiles.append(xt)

            def ctile_ap(t):  # stationary [KE, 128] for basis tile t
                c = ctiles[t * BAS_TILE // C_CHUNK]
                o = (t * BAS_TILE) % C_CHUNK
                return c[:, o:o + BAS_TILE]

            for s in range(n_slices):
                xt = xtiles[s * OBS_SLICE // X_CHUNK]
                xo = (s * OBS_SLICE) % X_CHUNK
                rhs = xt[:, xo:xo + OBS_SLICE]

                z = zpool.tile([1, OBS_SLICE], f32)
                t0 = 0
                for bi, bsz in enumerate(BLOCK_SIZES):
                    pool = gApool if bsz == 4 else gBpool
                    g = pool.tile([BAS_TILE, bsz * OBS_SLICE], f32)
                    for k in range(bsz):
                        nc.tensor.matmul(
                            g[:, k * OBS_SLICE:(k + 1) * OBS_SLICE],
                            lhsT=ctile_ap(t0 + k),
                            rhs=rhs,
                            start=True,
                            stop=True,
                        )
                    phi = ppool.tile([BAS_TILE, 4 * OBS_SLICE], f32)
                    nc.scalar.activation(
                        phi[:, :bsz * OBS_SLICE], g[:],
                        mybir.ActivationFunctionType.Exp,
                    )
                    for k in range(bsz):
                        nc.tensor.matmul(
                            z[:],
                            lhsT=wsb[:, t0 + k:t0 + k + 1],
                            rhs=phi[:, k * OBS_SLICE:(k + 1) * OBS_SLICE],
                            start=(t0 + k == 0),
                            stop=(t0 + k == n_btiles - 1),
                        )
                    t0 += bsz

                # sigmoid(z + b) = 0.5 + 0.5*tanh(0.5*z + 0.5*b)
                th = tpool.tile([1, OBS_SLICE], f32)
                nc.scalar.activation(
                    th[:], z[:], mybir.ActivationFunctionType.Tanh,
                    bias=hb[:], scale=0.5,
                )
                osb = tpool.tile([1, OBS_SLICE], f32)
                nc.vector.tensor_scalar(
                    out=osb[:], in0=th[:], scalar1=0.5, scalar2=0.5,
                    op0=mybir.AluOpType.mult, op1=mybir.AluOpType.add,
                )
                nc.sync.dma_start(
                    out=out_d.ap()[:, s * OBS_SLICE:(s + 1) * OBS_SLICE],
                    in_=osb[:],
                )

    nc.compile()
    return nc


def _get_program():
    if "nc" not in _prog_cache:
        _prog_cache["nc"] = _build_program()
    return _prog_cache["nc"]


def _prep_inputs(x, x_basis, w, b):
    x = np.asarray(x, dtype=np.float32)
    x_basis = np.asarray(x_basis, dtype=np.float32)
    w = np.asarray(w, dtype=np.float32).reshape(-1)
    b = np.asarray(b, dtype=np.float32).reshape(-1)

    cte = np.empty((KE, N_BASIS), dtype=np.float32)
    cte[:M_FEAT] = 2.0 * x_basis.T
    cte[M_FEAT] = -1.0
    cte[M_FEAT + 1] = -np.sum(x_basis * x_basis, axis=1)
    cte = np.ascontiguousarray(cte)

    wsb = np.ascontiguousarray(
        w.reshape(N_BASIS // BAS_TILE, BAS_TILE).T.astype(np.float32)
    )
    hb = np.array([[0.5 * b[0]]], dtype=np.float32)

    in_maps = []
    for c in range(N_CORES):
        xs = x[c * K_SHARD:(c + 1) * K_SHARD]
        xte = np.empty((KE, K_SHARD), dtype=np.float32)
        xte[:M_FEAT] = xs.T
        xte[M_FEAT] = np.sum(xs * xs, axis=1)
        xte[M_FEAT + 1] = 1.0
        in_maps.append({
            "xte": np.ascontiguousarray(xte),
            "cte": cte,
            "wsb": wsb,
            "hb": hb,
        })
    return in_maps


LAST_EXEC_NS = None


def kernel(x, x_basis, w, b):
    global LAST_EXEC_NS
    import os

    from concourse.bass_utils import run_bass_kernel_spmd

    nc = _get_program()
    in_maps = _prep_inputs(x, x_basis, w, b)

    trace = bool(os.environ.get("RBF_TRACE"))
    kwargs = {}
    if trace:
        tmpdir = os.environ.get("RBF_TRACE_DIR") or None
        kwargs = {"trace": True, "tmpdir": tmpdir}
    res = run_bass_kernel_spmd(nc, in_maps, list(range(N_CORES)), **kwargs)
    LAST_EXEC_NS = res.exec_time_ns

    out = np.concatenate([res.results[c]["out"][0] for c in range(N_CORES)])
    return out.reshape(K_FULL, 1).astype(np.float32)


# revision 2
# speedup vs baseline: 1.1205x; 1.1205x over previous
"""Trainium2 Bass kernel for nn_LogisticRegressionRBF.

reference:
    sq[i,j] = ||x_i||^2 + ||c_j||^2 - 2 x_i.c_j     (K=65536 obs, N=4096 basis)
    out     = sigmoid(exp(-sq) @ w.T + b)           [K, 1]

Strategy (data-parallel over obs across 8 cores, 8192 obs/core):
  * Basis centers are permuted so the ones with w_j >= 0 come first
    (n_pos of them), and ln|w_j| is folded into the augmented matmul, so
    exp directly produces |w_j| * exp(-sq):
       stationary xte[:, i] = [x_i (64) ; ||x_i||^2 ; 1]       (66 rows)
       moving     cte[:, j] = [2*c_j   ; -1 ; ln|w_j| - ||c_j||^2]
       G[i, j]  = xte.T @ cte = -sq[i, j] + ln|w_j|   (PSUM, obs on partitions)
    fp32 matmul runs LOW_HIGH (2 passes) but with obs stationary there are
    only 512 matmuls and 64 weight loads per core -> PE ~220us.
  * ACT exp on [128, 2048] PSUM ping-pong blocks WITH accum_out: the free-dim
    sum (= weighted reduction over basis) comes out of the same instruction.
    Sign-pure input segments -> separate accumulator columns for the w>=0 /
    w<0 groups; DVE combines them: z = sum(pos cols) - sum(neg cols).
  * sigmoid via tanh (same ACT table set as exp -> no table reload):
       sigmoid(z + b) = 0.5 + 0.5 * tanh(0.5*z + 0.5*b)
    batched once over [128, 64] at the end.
Output layout: zout[p, o] = out for obs o*128 + p; host transposes.
PSUM budget: two ping-pong blocks of 4 banks = 8.
"""

import sys

if "/opt/trn_rl_repo" not in sys.path:
    sys.path.insert(0, "/opt/trn_rl_repo")

import numpy as np

K_FULL, N_BASIS, M_FEAT = 65536, 4096, 64
N_CORES = 8
K_SHARD = K_FULL // N_CORES  # 8192
KE = M_FEAT + 2  # 66 augmented contraction rows
OBS_TILE = 128  # obs per stationary tile / psum partitions
BAS_SLICE = 512  # matmul moving free dim (one PSUM bank of fp32)
BLOCK = 2048  # basis cols per ACT block (4 PSUM banks)

_prog_cache: dict = {}


def _segments(n_pos):
    """Per ACT block: list of (lo, hi, col, is_pos) sign-pure segments in
    global permuted-basis coords; accumulator columns are assigned in order,
    positives first."""
    segs = []
    col = 0
    for lo in range(0, N_BASIS, BLOCK):
        hi = lo + BLOCK
        if n_pos <= lo:
            segs.append((lo, hi, None, False))
        elif n_pos >= hi:
            segs.append((lo, hi, None, True))
        else:
            segs.append((lo, n_pos, None, True))
            segs.append((n_pos, hi, None, False))
    segs.sort(key=lambda s: (not s[3], s[0]))
    segs = [(lo, hi, i, pos) for i, (lo, hi, _, pos) in enumerate(segs)]
    n_pos_cols = sum(1 for s in segs if s[3])
    segs.sort(key=lambda s: s[0])
    return segs, n_pos_cols


def _build_program(n_pos):
    import concourse.bacc as bacc
    import concourse.mybir as mybir
    import concourse.tile as tile

    f32 = mybir.dt.float32
    nc = bacc.Bacc("TRN2", target_bir_lowering=False, debug=False,
                   num_devices=N_CORES)

    xte_d = nc.dram_tensor("xte", [KE, K_SHARD], f32, kind="ExternalInput")
    cte_d = nc.dram_tensor("cte", [KE, N_BASIS], f32, kind="ExternalInput")
    hb_d = nc.dram_tensor("hb", [OBS_TILE, 1], f32, kind="ExternalInput")
    n_otiles = K_SHARD // OBS_TILE  # 64
    out_d = nc.dram_tensor("out", [OBS_TILE, n_otiles], f32,
                           kind="ExternalOutput")

    segs, n_pos_cols = _segments(n_pos)
    n_cols = len(segs)
    X_CHUNK = 2048  # xte DMA chunk (cols)
    C_CHUNK = 2048  # cte DMA chunk (cols)

    with tile.TileContext(nc) as tc:
        with (
            tc.tile_pool(name="const", bufs=1) as const,
            tc.tile_pool(name="xch", bufs=K_SHARD // X_CHUNK) as xpool,
            tc.tile_pool(name="cch", bufs=N_BASIS // C_CHUNK) as cpool,
            tc.tile_pool(name="phi", bufs=2) as ppool,
            tc.tile_pool(name="acc", bufs=3) as apool,
            tc.tile_pool(name="red", bufs=2) as rpool,
            tc.tile_pool(name="gA", bufs=1, space="PSUM") as gApool,
            tc.tile_pool(name="gB", bufs=1, space="PSUM") as gBpool,
        ):
            hb = const.tile([OBS_TILE, 1], f32)
            nc.sync.dma_start(out=hb[:], in_=hb_d.ap())

            ctiles = []
            for j in range(N_BASIS // C_CHUNK):
                ct = cpool.tile([KE, C_CHUNK], f32)
                nc.sync.dma_start(
                    out=ct[:], in_=cte_d.ap()[:, j * C_CHUNK:(j + 1) * C_CHUNK]
                )
                ctiles.append(ct)

            xtiles = []
            for j in range(K_SHARD // X_CHUNK):
                xt = xpool.tile([KE, X_CHUNK], f32)
                nc.sync.dma_start(
                    out=xt[:], in_=xte_d.ap()[:, j * X_CHUNK:(j + 1) * X_CHUNK]
                )
                xtiles.append(xt)

            zbuf = const.tile([OBS_TILE, n_otiles], f32)

            for o in range(n_otiles):
                xt = xtiles[o * OBS_TILE // X_CHUNK]
                xo = (o * OBS_TILE) % X_CHUNK
                lhsT = xt[:, xo:xo + OBS_TILE]

                acc = apool.tile([OBS_TILE, n_cols], f32)
                for b in range(N_BASIS // BLOCK):
                    pool = gApool if b % 2 == 0 else gBpool
                    g = pool.tile([OBS_TILE, BLOCK], f32)
                    ct = ctiles[b * BLOCK // C_CHUNK]
                    co = (b * BLOCK) % C_CHUNK
                    for k in range(BLOCK // BAS_SLICE):
                        nc.tensor.matmul(
                            g[:, k * BAS_SLICE:(k + 1) * BAS_SLICE],
                            lhsT=lhsT,
                            rhs=ct[:, co + k * BAS_SLICE:
                                   co + (k + 1) * BAS_SLICE],
                            start=True,
                            stop=True,
                        )
                    phi = ppool.tile([OBS_TILE, BLOCK], f32)
                    for lo, hi, col, _pos in segs:
                        if lo >= (b + 1) * BLOCK or hi <= b * BLOCK:
                            continue
                        s0 = lo - b * BLOCK
                        s1 = hi - b * BLOCK
                        nc.scalar.activation(
                            phi[:, s0:s1], g[:, s0:s1],
                            mybir.ActivationFunctionType.Exp,
                            accum_out=acc[:, col:col + 1],
                        )

                # z = sum(pos cols) - sum(neg cols)
                if n_pos_cols == 0:
                    nc.vector.reduce_sum(
                        zbuf[:, o:o + 1], acc[:], axis=mybir.AxisListType.X,
                        negate=True,
                    )
                elif n_pos_cols == n_cols:
                    nc.vector.reduce_sum(
                        zbuf[:, o:o + 1], acc[:], axis=mybir.AxisListType.X,
                    )
                else:
                    zp = rpool.tile([OBS_TILE, 1], f32)
                    zn = rpool.tile([OBS_TILE, 1], f32)
                    nc.vector.reduce_sum(
                        zp[:], acc[:, 0:n_pos_cols], axis=mybir.AxisListType.X,
                    )
                    nc.vector.reduce_sum(
                        zn[:], acc[:, n_pos_cols:n_cols],
                        axis=mybir.AxisListType.X,
                    )
                    nc.vector.tensor_sub(zbuf[:, o:o + 1], zp[:], zn[:])

            # sigmoid(z + b) = 0.5 + 0.5*tanh(0.5*z + 0.5*b), batched
            th = const.tile([OBS_TILE, n_otiles], f32)
            nc.scalar.activation(
                th[:], zbuf[:], mybir.ActivationFunctionType.Tanh,
                bias=hb[:], scale=0.5,
            )
            osb = const.tile([OBS_TILE, n_otiles], f32)
            nc.vector.tensor_scalar(
                out=osb[:], in0=th[:], scalar1=0.5, scalar2=0.5,
                op0=mybir.AluOpType.mult, op1=mybir.AluOpType.add,
            )
            nc.sync.dma_start(out=out_d.ap(), in_=osb[:])

    nc.compile()
    return nc


def _get_program(n_pos):
    if n_pos not in _prog_cache:
        _prog_cache[n_pos] = _build_program(n_pos)
    return _prog_cache[n_pos]


def _prep_inputs(x, x_basis, w, b):
    x = np.asarray(x, dtype=np.float32)
    x_basis = np.asarray(x_basis, dtype=np.float32)
    w = np.asarray(w, dtype=np.float32).reshape(-1)
    b = np.asarray(b, dtype=np.float32).reshape(-1)

    pos = w >= 0
    perm = np.concatenate([np.nonzero(pos)[0], np.nonzero(~pos)[0]])
    n_pos = int(pos.sum())

    cb = x_basis[perm]
    lw = np.log(np.maximum(np.abs(w[perm]), 1e-35)).astype(np.float32)
    cte = np.empty((KE, N_BASIS), dtype=np.float32)
    cte[:M_FEAT] = 2.0 * cb.T
    cte[M_FEAT] = -1.0
    cte[M_FEAT + 1] = lw - np.sum(cb * cb, axis=1)
    cte = np.ascontiguousarray(cte)

    hb = np.full((OBS_TILE, 1), 0.5 * b[0], dtype=np.float32)

    in_maps = []
    for c in range(N_CORES):
        xs = x[c * K_SHARD:(c + 1) * K_SHARD]
        xte = np.empty((KE, K_SHARD), dtype=np.float32)
        xte[:M_FEAT] = xs.T
        xte[M_FEAT] = np.sum(xs * xs, axis=1)
        xte[M_FEAT + 1] = 1.0
        in_maps.append({
            "xte": np.ascontiguousarray(xte),
            "cte": cte,
            "hb": hb,
        })
    return in_maps, n_pos


LAST_EXEC_NS = None


def kernel(x, x_basis, w, b):
    global LAST_EXEC_NS
    import os

    from concourse.bass_utils import run_bass_kernel_spmd

    in_maps, n_pos = _prep_inputs(x, x_basis, w, b)
    nc = _get_program(n_pos)

    trace = bool(os.environ.get("RBF_TRACE"))
    kwargs = {}
    if trace:
        tmpdir = os.environ.get("RBF_TRACE_DIR") or None
        kwargs = {"trace": True, "tmpdir": tmpdir}
    res = run_bass_kernel_spmd(nc, in_maps, list(range(N_CORES)), **kwargs)
    LAST_EXEC_NS = res.exec_time_ns

    # zout[p, o] = out for obs o*128 + p
    out = np.concatenate(
        [res.results[c]["out"].T.reshape(K_SHARD) for c in range(N_CORES)]
    )
    return out.reshape(K_FULL, 1).astype(np.float32)


# revision 3
# speedup vs baseline: 1.6135x; 1.4400x over previous
"""Trainium2 Bass kernel for nn_LogisticRegressionRBF.

reference:
    sq[i,j] = ||x_i||^2 + ||c_j||^2 - 2 x_i.c_j     (K=65536 obs, N=4096 basis)
    out     = sigmoid(exp(-sq) @ w.T + b)           [K, 1]

Strategy (data-parallel over obs across 8 cores, 8192 obs/core):
  * Basis centers are permuted so the ones with w_j >= 0 come first
    (n_pos of them), and ln|w_j| is folded into the matmul, so exp directly
    produces |w_j| * exp(-sq); the free-dim sum of that IS the weighted
    reduction, which the ACT engine computes for free via accum_out.
  * fp32 PE matmuls are slow on TRN2 (LOW_HIGH double pass + serial
    non-FWL weight loads), so the distance matmul runs as a compensated
    bf16 pair accumulating into the same PSUM slice:
      main [67 rows]:  Ah(x) . Bh(2c)  - bf16(x2) + v_h + v_l
                        (v = ln|w| - ||c||^2 split into two bf16 rows)
      corr [128 rows]: Al(x) . Bh(2c)  +  Ah(x) . Bl(2c)
    and the fp32 remainder of ||x||^2 is applied exactly as the ACT
    per-partition bias:  exp(G + (-x2_lo)).  Total argument error ~4e-4.
  * ACT exp runs in-place on [128, 2048] PSUM ping-pong blocks with
    accum_out -> sign-pure accumulator columns; DVE combines:
    z = sum(pos cols) - sum(neg cols).
  * sigmoid via tanh (same ACT table set as exp -> no table reload):
       sigmoid(z + b) = 0.5 + 0.5 * tanh(0.5*z + 0.5*b), batched [128, 64].
Output layout: zout[p, o] = out for obs o*128 + p; host transposes.
"""

import sys

if "/opt/trn_rl_repo" not in sys.path:
    sys.path.insert(0, "/opt/trn_rl_repo")

import numpy as np

K_FULL, N_BASIS, M_FEAT = 65536, 4096, 64
N_CORES = 8
K_SHARD = K_FULL // N_CORES  # 8192
KM = M_FEAT + 3  # 67 main rows
KC = 2 * M_FEAT  # 128 correction rows
OBS_TILE = 128  # obs per stationary tile / psum partitions
BAS_SLICE = 512  # matmul moving free dim (one PSUM bank of fp32)
BLOCK = 2048  # basis cols per ACT block (4 PSUM banks)

_prog_cache: dict = {}


def _segments(n_pos):
    """Sign-pure (lo, hi, col, is_pos) segments per ACT block; accumulator
    columns assigned positives-first."""
    segs = []
    for lo in range(0, N_BASIS, BLOCK):
        hi = lo + BLOCK
        if n_pos <= lo:
            segs.append((lo, hi, None, False))
        elif n_pos >= hi:
            segs.append((lo, hi, None, True))
        else:
            segs.append((lo, n_pos, None, True))
            segs.append((n_pos, hi, None, False))
    segs.sort(key=lambda s: (not s[3], s[0]))
    segs = [(lo, hi, i, pos) for i, (lo, hi, _, pos) in enumerate(segs)]
    n_pos_cols = sum(1 for s in segs if s[3])
    segs.sort(key=lambda s: s[0])
    return segs, n_pos_cols


def _build_program(n_pos):
    import concourse.bacc as bacc
    import concourse.mybir as mybir
    import concourse.tile as tile

    f32 = mybir.dt.float32
    bf16 = mybir.dt.bfloat16
    nc = bacc.Bacc("TRN2", target_bir_lowering=False, debug=False,
                   num_devices=N_CORES)

    am_d = nc.dram_tensor("am", [KM, K_SHARD], bf16, kind="ExternalInput")
    ac_d = nc.dram_tensor("ac", [KC, K_SHARD], bf16, kind="ExternalInput")
    bm_d = nc.dram_tensor("bm", [KM, N_BASIS], bf16, kind="ExternalInput")
    bc_d = nc.dram_tensor("bc", [KC, N_BASIS], bf16, kind="ExternalInput")
    n_otiles = K_SHARD // OBS_TILE  # 64
    x2l_d = nc.dram_tensor("x2l", [OBS_TILE, n_otiles], f32,
                           kind="ExternalInput")
    hb_d = nc.dram_tensor("hb", [OBS_TILE, 1], f32, kind="ExternalInput")
    out_d = nc.dram_tensor("out", [OBS_TILE, n_otiles], f32,
                           kind="ExternalOutput")

    segs, n_pos_cols = _segments(n_pos)
    n_cols = len(segs)
    X_CHUNK = 2048
    C_CHUNK = 2048

    with tile.TileContext(nc) as tc:
        with (
            tc.tile_pool(name="const", bufs=1) as const,
            tc.tile_pool(name="amch", bufs=K_SHARD // X_CHUNK) as ampool,
            tc.tile_pool(name="acch", bufs=K_SHARD // X_CHUNK) as acpool,
            tc.tile_pool(name="bmch", bufs=N_BASIS // C_CHUNK) as bmpool,
            tc.tile_pool(name="bcch", bufs=N_BASIS // C_CHUNK) as bcpool,
            tc.tile_pool(name="acc", bufs=3) as apool,
            tc.tile_pool(name="red", bufs=2) as rpool,
            tc.tile_pool(name="gA", bufs=1, space="PSUM") as gApool,
            tc.tile_pool(name="gB", bufs=1, space="PSUM") as gBpool,
        ):
            hb = const.tile([OBS_TILE, 1], f32)
            nc.sync.dma_start(out=hb[:], in_=hb_d.ap())
            x2l = const.tile([OBS_TILE, n_otiles], f32)
            nc.sync.dma_start(out=x2l[:], in_=x2l_d.ap())

            def load_chunks(pool, dram, rows, total, chunk):
                tiles = []
                for j in range(total // chunk):
                    t = pool.tile([rows, chunk], bf16)
                    nc.sync.dma_start(
                        out=t[:], in_=dram.ap()[:, j * chunk:(j + 1) * chunk]
                    )
                    tiles.append(t)
                return tiles

            amtiles = load_chunks(ampool, am_d, KM, K_SHARD, X_CHUNK)
            actiles = load_chunks(acpool, ac_d, KC, K_SHARD, X_CHUNK)
            bmtiles = load_chunks(bmpool, bm_d, KM, N_BASIS, C_CHUNK)
            bctiles = load_chunks(bcpool, bc_d, KC, N_BASIS, C_CHUNK)

            zbuf = const.tile([OBS_TILE, n_otiles], f32)

            for o in range(n_otiles):
                xj, xo = o * OBS_TILE // X_CHUNK, (o * OBS_TILE) % X_CHUNK
                lhs_m = amtiles[xj][:, xo:xo + OBS_TILE]
                lhs_c = actiles[xj][:, xo:xo + OBS_TILE]

                acc = apool.tile([OBS_TILE, n_cols], f32)
                gtiles = []
                for b in range(N_BASIS // BLOCK):
                    pool = gApool if b % 2 == 0 else gBpool
                    g = pool.tile([OBS_TILE, BLOCK], f32)
                    gtiles.append(g)
                    for k in range(BLOCK // BAS_SLICE):
                        col = b * BLOCK + k * BAS_SLICE
                        cj, co = col // C_CHUNK, col % C_CHUNK
                        nc.tensor.matmul(
                            g[:, k * BAS_SLICE:(k + 1) * BAS_SLICE],
                            lhsT=lhs_m,
                            rhs=bmtiles[cj][:, co:co + BAS_SLICE],
                            start=True, stop=False,
                        )
                    for k in range(BLOCK // BAS_SLICE):
                        col = b * BLOCK + k * BAS_SLICE
                        cj, co = col // C_CHUNK, col % C_CHUNK
                        nc.tensor.matmul(
                            g[:, k * BAS_SLICE:(k + 1) * BAS_SLICE],
                            lhsT=lhs_c,
                            rhs=bctiles[cj][:, co:co + BAS_SLICE],
                            start=False, stop=True,
                        )
                    for lo, hi, cidx, _pos in segs:
                        if lo >= (b + 1) * BLOCK or hi <= b * BLOCK:
                            continue
                        s0, s1 = lo - b * BLOCK, hi - b * BLOCK
                        nc.scalar.activation(
                            g[:, s0:s1], g[:, s0:s1],
                            mybir.ActivationFunctionType.Exp,
                            bias=x2l[:, o:o + 1],
                            accum_out=acc[:, cidx:cidx + 1],
                        )

                if n_pos_cols == 0:
                    nc.vector.reduce_sum(
                        zbuf[:, o:o + 1], acc[:], axis=mybir.AxisListType.X,
                        negate=True,
                    )
                elif n_pos_cols == n_cols:
                    nc.vector.reduce_sum(
                        zbuf[:, o:o + 1], acc[:], axis=mybir.AxisListType.X,
                    )
                else:
                    zp = rpool.tile([OBS_TILE, 1], f32)
                    zn = rpool.tile([OBS_TILE, 1], f32)
                    nc.vector.reduce_sum(
                        zp[:], acc[:, 0:n_pos_cols], axis=mybir.AxisListType.X,
                    )
                    nc.vector.reduce_sum(
                        zn[:], acc[:, n_pos_cols:n_cols],
                        axis=mybir.AxisListType.X,
                    )
                    nc.vector.tensor_sub(zbuf[:, o:o + 1], zp[:], zn[:])

            # sigmoid(z + b) = 0.5 + 0.5*tanh(0.5*z + 0.5*b), batched
            th = const.tile([OBS_TILE, n_otiles], f32)
            nc.scalar.activation(
                th[:], zbuf[:], mybir.ActivationFunctionType.Tanh,
                bias=hb[:], scale=0.5,
            )
            osb = const.tile([OBS_TILE, n_otiles], f32)
            nc.vector.tensor_scalar(
                out=osb[:], in0=th[:], scalar1=0.5, scalar2=0.5,
                op0=mybir.AluOpType.mult, op1=mybir.AluOpType.add,
            )
            nc.sync.dma_start(out=out_d.ap(), in_=osb[:])

    nc.compile()
    return nc


def _get_program(n_pos):
    if n_pos not in _prog_cache:
        _prog_cache[n_pos] = _build_program(n_pos)
    return _prog_cache[n_pos]


def _bf16(a):
    import ml_dtypes

    return a.astype(ml_dtypes.bfloat16)


def _prep_inputs(x, x_basis, w, b):
    x = np.asarray(x, dtype=np.float32)
    x_basis = np.asarray(x_basis, dtype=np.float32)
    w = np.asarray(w, dtype=np.float32).reshape(-1)
    b = np.asarray(b, dtype=np.float32).reshape(-1)

    pos = w >= 0
    perm = np.concatenate([np.nonzero(pos)[0], np.nonzero(~pos)[0]])
    n_pos = int(pos.sum())

    cb2 = 2.0 * x_basis[perm].T  # [64, N]
    lw = np.log(np.maximum(np.abs(w[perm]), 1e-35)).astype(np.float32)
    v = lw - np.sum(x_basis[perm] * x_basis[perm], axis=1)
    v_h = _bf16(v)
    v_l = _bf16(v - v_h.astype(np.float32))
    bh = _bf16(cb2)
    bl = _bf16(cb2 - bh.astype(np.float32))

    import ml_dtypes

    bm = np.empty((KM, N_BASIS), dtype=ml_dtypes.bfloat16)
    bm[:M_FEAT] = bh
    bm[M_FEAT] = _bf16(np.full(N_BASIS, -1.0, np.float32))
    bm[M_FEAT + 1] = v_h
    bm[M_FEAT + 2] = v_l
    bc = np.ascontiguousarray(np.concatenate([bh, bl], axis=0))

    hb = np.full((OBS_TILE, 1), 0.5 * b[0], dtype=np.float32)
    n_otiles = K_SHARD // OBS_TILE

    in_maps = []
    for c in range(N_CORES):
        xs = x[c * K_SHARD:(c + 1) * K_SHARD]
        xt = xs.T  # [64, 8192]
        x2 = np.sum(xs * xs, axis=1)
        x2h = _bf16(x2)
        x2l = -(x2 - x2h.astype(np.float32))  # fp32 remainder, negated
        ah = _bf16(xt)
        al = _bf16(xt - ah.astype(np.float32))

        am = np.empty((KM, K_SHARD), dtype=ml_dtypes.bfloat16)
        am[:M_FEAT] = ah
        am[M_FEAT] = x2h
        am[M_FEAT + 1] = _bf16(np.ones(K_SHARD, np.float32))
        am[M_FEAT + 2] = am[M_FEAT + 1]
        ac = np.ascontiguousarray(np.concatenate([al, ah], axis=0))

        in_maps.append({
            "am": np.ascontiguousarray(am),
            "ac": ac,
            "bm": np.ascontiguousarray(bm),
            "bc": bc,
            "x2l": np.ascontiguousarray(
                x2l.reshape(n_otiles, OBS_TILE).T.astype(np.float32)
            ),
            "hb": hb,
        })
    return in_maps, n_pos


LAST_EXEC_NS = None


def kernel(x, x_basis, w, b):
    global LAST_EXEC_NS
    import os

    from concourse.bass_utils import run_bass_kernel_spmd

    in_maps, n_pos = _prep_inputs(x, x_basis, w, b)
    nc = _get_program(n_pos)

    trace = bool(os.environ.get("RBF_TRACE"))
    kwargs = {}
    if trace:
        tmpdir = os.environ.get("RBF_TRACE_DIR") or None
        kwargs = {"trace": True, "tmpdir": tmpdir}
    res = run_bass_kernel_spmd(nc, in_maps, list(range(N_CORES)), **kwargs)
    LAST_EXEC_NS = res.exec_time_ns

    # zout[p, o] = out for obs o*128 + p
    out = np.concatenate(
        [res.results[c]["out"].T.reshape(K_SHARD) for c in range(N_CORES)]
    )
    return out.reshape(K_FULL, 1).astype(np.float32)


# revision 9
# speedup vs baseline: 1.7383x; 1.0774x over previous
"""Trainium2 Bass kernel for nn_LogisticRegressionRBF.

reference:
    sq[i,j] = ||x_i||^2 + ||c_j||^2 - 2 x_i.c_j     (K=65536 obs, N=4096 basis)
    out     = sigmoid(exp(-sq) @ w.T + b)           [K, 1]

Strategy (data-parallel over obs across 8 cores, 8192 obs/core):
  * Basis centers are permuted so the ones with w_j >= 0 come first
    (n_pos of them), and ln|w_j| is folded into the matmul, so exp directly
    produces |w_j| * exp(-sq); the free-dim sum of that IS the weighted
    reduction, which the ACT engine computes for free via accum_out.
  * fp32 PE matmuls are slow on TRN2 (LOW_HIGH double pass + serial
    non-FWL weight loads), so the distance matmul runs as a compensated
    bf16 pair accumulating into the same PSUM slice:
      main [67 rows]:  Ah(x) . Bh(2c)  - bf16(x2) + v_h + v_l
                        (v = ln|w| - ||c||^2 split into two bf16 rows)
      corr [128 rows]: Al(x) . Bh(2c)  +  Ah(x) . Bl(2c)
    and the fp32 remainder of ||x||^2 is applied exactly as the ACT
    per-partition bias:  exp(G + (-x2_lo)).  Total argument error ~4e-4.
  * ACT exp runs in-place on [128, 2048] PSUM ping-pong blocks with
    accum_out -> sign-pure accumulator columns; DVE combines:
    z = sum(pos cols) - sum(neg cols).
  * sigmoid via tanh (same ACT table set as exp -> no table reload):
       sigmoid(z + b) = 0.5 + 0.5 * tanh(0.5*z + 0.5*b), batched [128, 64].
Output layout: zout[p, o] = out for obs o*128 + p; host transposes.
"""

import sys

if "/opt/trn_rl_repo" not in sys.path:
    sys.path.insert(0, "/opt/trn_rl_repo")

import numpy as np

K_FULL, N_BASIS, M_FEAT = 65536, 4096, 64
N_CORES = 8
K_SHARD = K_FULL // N_CORES  # 8192
KM = M_FEAT + 3  # 67 main rows
KC = 2 * M_FEAT  # 128 correction rows
OBS_TILE = 128  # obs per stationary tile / psum partitions
BAS_SLICE = 512  # matmul moving free dim (one PSUM bank of fp32)
BLOCK = 2048  # basis cols per ACT block (4 PSUM banks)

_prog_cache: dict = {}


def _blocks(n_pos):
    """Sign-pure PSUM blocks (lo, hi, col): the basis axis is cut exactly at
    n_pos, each side chunked into <=BLOCK pieces.  Emitted smallest-first so
    tiny remainder blocks lead the pipeline instead of straggling.
    Accumulator columns: positives get [0, n_pos_cols)."""
    blocks = []
    for lo, hi, pos in [(0, n_pos, True), (n_pos, N_BASIS, False)]:
        cur = lo
        while cur < hi:
            sz = min(BLOCK, hi - cur)
            # leave the remainder as its own block
            if (hi - cur) % BLOCK and hi - cur > BLOCK:
                sz = (hi - cur) % BLOCK
            blocks.append([cur, cur + sz, pos])
            cur += sz
    blocks.sort(key=lambda b: b[1] - b[0])
    n_pos_cols = sum(1 for b in blocks if b[2])
    pc, nc_ = 0, n_pos_cols
    out = []
    for lo, hi, pos in blocks:
        if pos:
            out.append((lo, hi, pc))
            pc += 1
        else:
            out.append((lo, hi, nc_))
            nc_ += 1
    return out, n_pos_cols


def _build_program(n_pos):
    import concourse.bacc as bacc
    import concourse.mybir as mybir
    import concourse.tile as tile

    f32 = mybir.dt.float32
    bf16 = mybir.dt.bfloat16
    nc = bacc.Bacc("TRN2", target_bir_lowering=False, debug=False,
                   num_devices=N_CORES)

    am_d = nc.dram_tensor("am", [KM, K_SHARD], bf16, kind="ExternalInput")
    ac_d = nc.dram_tensor("ac", [KC, K_SHARD], bf16, kind="ExternalInput")
    bm_d = nc.dram_tensor("bm", [KM, N_BASIS], bf16, kind="ExternalInput")
    bc_d = nc.dram_tensor("bc", [KC, N_BASIS], bf16, kind="ExternalInput")
    n_otiles = K_SHARD // OBS_TILE  # 64
    x2l_d = nc.dram_tensor("x2l", [OBS_TILE, n_otiles], f32,
                           kind="ExternalInput")
    hb_d = nc.dram_tensor("hb", [OBS_TILE, 1], f32, kind="ExternalInput")
    out_d = nc.dram_tensor("out", [OBS_TILE, n_otiles], f32,
                           kind="ExternalOutput")

    blocks, n_pos_cols = _blocks(n_pos)
    n_cols = len(blocks)
    X_CHUNK = 2048
    C_CHUNK = 2048

    with tile.TileContext(nc) as tc:
        with (
            tc.tile_pool(name="const", bufs=1) as const,
            tc.tile_pool(name="amch", bufs=K_SHARD // X_CHUNK) as ampool,
            tc.tile_pool(name="acch", bufs=K_SHARD // X_CHUNK) as acpool,
            tc.tile_pool(name="bmch", bufs=N_BASIS // C_CHUNK) as bmpool,
            tc.tile_pool(name="bcch", bufs=N_BASIS // C_CHUNK) as bcpool,
            tc.tile_pool(name="acc", bufs=3) as apool,
            tc.tile_pool(name="red", bufs=2) as rpool,
            tc.tile_pool(name="gps", bufs=2, space="PSUM") as gpool,
        ):
            hb = const.tile([OBS_TILE, 1], f32)
            nc.sync.dma_start(out=hb[:], in_=hb_d.ap())
            x2l = const.tile([OBS_TILE, n_otiles], f32)
            nc.sync.dma_start(out=x2l[:], in_=x2l_d.ap())

            dma_engines = [nc.sync, nc.gpsimd, nc.scalar, nc.gpsimd]
            dma_rr = [0]

            def load_chunks(pool, dram, rows, total, chunk):
                tiles = []
                for j in range(total // chunk):
                    t = pool.tile([rows, chunk], bf16)
                    eng = dma_engines[dma_rr[0] % len(dma_engines)]
                    dma_rr[0] += 1
                    eng.dma_start(
                        out=t[:], in_=dram.ap()[:, j * chunk:(j + 1) * chunk]
                    )
                    tiles.append(t)
                return tiles

            # bm/bc as single tiles: matmul moving slices cut at arbitrary
            # (n_pos-dependent) offsets and must not cross SBUF tile seams
            bm = bmpool.tile([KM, N_BASIS], bf16)
            bc = bcpool.tile([KC, N_BASIS], bf16)
            for j in range(N_BASIS // C_CHUNK):
                sl = slice(j * C_CHUNK, (j + 1) * C_CHUNK)
                dma_engines[j % 4].dma_start(out=bm[:, sl], in_=bm_d.ap()[:, sl])
                dma_engines[(j + 2) % 4].dma_start(
                    out=bc[:, sl], in_=bc_d.ap()[:, sl])
            amtiles = load_chunks(ampool, am_d, KM, K_SHARD, X_CHUNK)
            actiles = load_chunks(acpool, ac_d, KC, K_SHARD, X_CHUNK)

            zbuf = const.tile([OBS_TILE, n_otiles], f32)

            for o in range(n_otiles):
                xj, xo = o * OBS_TILE // X_CHUNK, (o * OBS_TILE) % X_CHUNK
                lhs_m = amtiles[xj][:, xo:xo + OBS_TILE]
                lhs_c = actiles[xj][:, xo:xo + OBS_TILE]

                acc = apool.tile([OBS_TILE, n_cols], f32)
                for lo, hi, cidx in blocks:
                    sz = hi - lo
                    g = gpool.tile([OBS_TILE, BLOCK], f32, tag="gps")
                    # PSUM-bank-aligned sub-slices relative to block start
                    cuts = [(c, min(sz, c + BAS_SLICE))
                            for c in range(0, sz, BAS_SLICE)]
                    for lhs, bsrc, start, stop in (
                        (lhs_m, bm, True, False),
                        (lhs_c, bc, False, True),
                    ):
                        for s0, s1 in cuts:
                            nc.tensor.matmul(
                                g[:, s0:s1],
                                lhsT=lhs,
                                rhs=bsrc[:, lo + s0:lo + s1],
                                start=start, stop=stop,
                            )
                    nc.scalar.activation(
                        g[:, 0:sz], g[:, 0:sz],
                        mybir.ActivationFunctionType.Exp,
                        bias=x2l[:, o:o + 1],
                        accum_out=acc[:, cidx:cidx + 1],
                    )

                if n_pos_cols == 0:
                    nc.vector.reduce_sum(
                        zbuf[:, o:o + 1], acc[:], axis=mybir.AxisListType.X,
                        negate=True,
                    )
                elif n_pos_cols == n_cols:
                    nc.vector.reduce_sum(
                        zbuf[:, o:o + 1], acc[:], axis=mybir.AxisListType.X,
                    )
                else:
                    zp = rpool.tile([OBS_TILE, 1], f32)
                    zn = rpool.tile([OBS_TILE, 1], f32)
                    nc.vector.reduce_sum(
                        zp[:], acc[:, 0:n_pos_cols], axis=mybir.AxisListType.X,
                    )
                    nc.vector.reduce_sum(
                        zn[:], acc[:, n_pos_cols:n_cols],
                        axis=mybir.AxisListType.X,
                    )
                    nc.vector.tensor_sub(zbuf[:, o:o + 1], zp[:], zn[:])

            # sigmoid(z + b) = 0.5 + 0.5*tanh(0.5*z + 0.5*b), batched
            th = const.tile([OBS_TILE, n_otiles], f32)
            nc.scalar.activation(
                th[:], zbuf[:], mybir.ActivationFunctionType.Tanh,
                bias=hb[:], scale=0.5,
            )
            osb = const.tile([OBS_TILE, n_otiles], f32)
            nc.vector.tensor_scalar(
                out=osb[:], in0=th[:], scalar1=0.5, scalar2=0.5,
                op0=mybir.AluOpType.mult, op1=mybir.AluOpType.add,
            )
            nc.sync.dma_start(out=out_d.ap(), in_=osb[:])

    nc.compile()
    return nc


def _get_program(n_pos):
    if n_pos not in _prog_cache:
        _prog_cache[n_pos] = _build_program(n_pos)
    return _prog_cache[n_pos]


def _bf16(a):
    import ml_dtypes

    return a.astype(ml_dtypes.bfloat16)


def _prep_inputs(x, x_basis, w, b):
    x = np.asarray(x, dtype=np.float32)
    x_basis = np.asarray(x_basis, dtype=np.float32)
    w = np.asarray(w, dtype=np.float32).reshape(-1)
    b = np.asarray(b, dtype=np.float32).reshape(-1)

    pos = w >= 0
    perm = np.concatenate([np.nonzero(pos)[0], np.nonzero(~pos)[0]])
    n_pos = int(pos.sum())

    cb2 = 2.0 * x_basis[perm].T  # [64, N]
    lw = np.log(np.maximum(np.abs(w[perm]), 1e-35)).astype(np.float32)
    v = lw - np.sum(x_basis[perm] * x_basis[perm], axis=1)
    v_h = _bf16(v)
    v_l = _bf16(v - v_h.astype(np.float32))
    bh = _bf16(cb2)
    bl = _bf16(cb2 - bh.astype(np.float32))

    import ml_dtypes

    bm = np.empty((KM, N_BASIS), dtype=ml_dtypes.bfloat16)
    bm[:M_FEAT] = bh
    bm[M_FEAT] = _bf16(np.full(N_BASIS, -1.0, np.float32))
    bm[M_FEAT + 1] = v_h
    bm[M_FEAT + 2] = v_l
    bc = np.ascontiguousarray(np.concatenate([bh, bl], axis=0))

    hb = np.full((OBS_TILE, 1), 0.5 * b[0], dtype=np.float32)
    n_otiles = K_SHARD // OBS_TILE

    in_maps = []
    for c in range(N_CORES):
        xs = x[c * K_SHARD:(c + 1) * K_SHARD]
        xt = xs.T  # [64, 8192]
        x2 = np.sum(xs * xs, axis=1)
        x2h = _bf16(x2)
        x2l = -(x2 - x2h.astype(np.float32))  # fp32 remainder, negated
        ah = _bf16(xt)
        al = _bf16(xt - ah.astype(np.float32))

        am = np.empty((KM, K_SHARD), dtype=ml_dtypes.bfloat16)
        am[:M_FEAT] = ah
        am[M_FEAT] = x2h
        am[M_FEAT + 1] = _bf16(np.ones(K_SHARD, np.float32))
        am[M_FEAT + 2] = am[M_FEAT + 1]
        ac = np.ascontiguousarray(np.concatenate([al, ah], axis=0))

        in_maps.append({
            "am": np.ascontiguousarray(am),
            "ac": ac,
            "bm": np.ascontiguousarray(bm),
            "bc": bc,
            "x2l": np.ascontiguousarray(
                x2l.reshape(n_otiles, OBS_TILE).T.astype(np.float32)
            ),
            "hb": hb,
        })
    return in_maps, n_pos


LAST_EXEC_NS = None


def kernel(x, x_basis, w, b):
    global LAST_EXEC_NS
    import os

    from concourse.bass_utils import run_bass_kernel_spmd

    in_maps, n_pos = _prep_inputs(x, x_basis, w, b)
    nc = _get_program(n_pos)

    trace = bool(os.environ.get("RBF_TRACE"))
    kwargs = {}
    if trace:
        tmpdir = os.environ.get("RBF_TRACE_DIR") or None
        kwargs = {"trace": True, "tmpdir": tmpdir}
    res = run_bass_kernel_spmd(nc, in_maps, list(range(N_CORES)), **kwargs)
    LAST_EXEC_NS = res.exec_time_ns

    # zout[p, o] = out for obs o*128 + p
    out = np.concatenate(
        [res.results[c]["out"].T.reshape(K_SHARD) for c in range(N_CORES)]
    )
    return out.reshape(K_FULL, 1).astype(np.float32)


# revision 14
# speedup vs baseline: 2.6555x; 1.5276x over previous
"""Trainium2 Bass kernel for nn_LogisticRegressionRBF.

reference:
    sq[i,j] = ||x_i||^2 + ||c_j||^2 - 2 x_i.c_j     (K=65536 obs, N=4096 basis)
    out     = sigmoid(exp(-sq) @ w.T + b)           [K, 1]

Strategy (data-parallel over obs across 8 cores, 8192 obs/core):
  * Basis centers are permuted so the ones with w_j >= 0 come first
    (n_pos of them), and ln|w_j| is folded into the matmul, so exp directly
    produces |w_j| * exp(-sq); the free-dim sum of that IS the weighted
    reduction, which the ACT engine computes for free via accum_out.
  * fp32 PE matmuls are slow on TRN2 (LOW_HIGH double pass + serial
    non-FWL weight loads), so the distance matmul runs as a compensated
    bf16 pair accumulating into the same PSUM slice:
      main [67 rows]:  Ah(x) . Bh(2c)  - bf16(x2) + v_h + v_l
                        (v = ln|w| - ||c||^2 split into two bf16 rows)
      corr [128 rows]: Al(x) . Bh(2c)  +  Ah(x) . Bl(2c)
    and the fp32 remainder of ||x||^2 is applied exactly as the ACT
    per-partition bias:  exp(G + (-x2_lo)).  Total argument error ~4e-4.
  * ACT exp runs in-place on [128, 2048] PSUM ping-pong blocks with
    accum_out -> sign-pure accumulator columns; DVE combines:
    z = sum(pos cols) - sum(neg cols).
  * sigmoid via tanh (same ACT table set as exp -> no table reload):
       sigmoid(z + b) = 0.5 + 0.5 * tanh(0.5*z + 0.5*b), batched [128, 64].
Output layout: zout[p, o] = out for obs o*128 + p; host transposes.
"""

import sys

if "/opt/trn_rl_repo" not in sys.path:
    sys.path.insert(0, "/opt/trn_rl_repo")

import numpy as np

K_FULL, N_BASIS, M_FEAT = 65536, 4096, 64
N_CORES = 8
K_SHARD = K_FULL // N_CORES  # 8192
KM = M_FEAT + 3  # 67 main rows
KC = 2 * M_FEAT  # 128 correction rows
OBS_TILE = 128  # obs per stationary tile / psum partitions
BAS_SLICE = 512  # matmul moving free dim (one PSUM bank of fp32)
BLOCK = 1024  # basis cols per ACT block (2 PSUM banks)
PSUM_BUFS = 4

_prog_cache: dict = {}


def _blocks(n_pos):
    """Sign-pure PSUM blocks (lo, hi, col): the basis axis is cut exactly at
    n_pos, each side chunked into <=BLOCK pieces.  Emitted smallest-first so
    tiny remainder blocks lead the pipeline instead of straggling.
    Accumulator columns: positives get [0, n_pos_cols)."""
    blocks = []
    for lo, hi, pos in [(0, n_pos, True), (n_pos, N_BASIS, False)]:
        cur = lo
        while cur < hi:
            sz = min(BLOCK, hi - cur)
            # leave the remainder as its own block
            if (hi - cur) % BLOCK and hi - cur > BLOCK:
                sz = (hi - cur) % BLOCK
            blocks.append([cur, cur + sz, pos])
            cur += sz
    blocks.sort(key=lambda b: b[1] - b[0])
    n_pos_cols = sum(1 for b in blocks if b[2])
    pc, nc_ = 0, n_pos_cols
    out = []
    for lo, hi, pos in blocks:
        if pos:
            out.append((lo, hi, pc))
            pc += 1
        else:
            out.append((lo, hi, nc_))
            nc_ += 1
    return out, n_pos_cols


def _build_program(n_pos):
    import concourse.bacc as bacc
    import concourse.mybir as mybir
    import concourse.tile as tile

    f32 = mybir.dt.float32
    bf16 = mybir.dt.bfloat16
    nc = bacc.Bacc("TRN2", target_bir_lowering=False, debug=False,
                   num_devices=N_CORES)

    am_d = nc.dram_tensor("am", [KM, K_SHARD], bf16, kind="ExternalInput")
    ac_d = nc.dram_tensor("ac", [KC, K_SHARD], bf16, kind="ExternalInput")
    bm_d = nc.dram_tensor("bm", [KM, N_BASIS], bf16, kind="ExternalInput")
    bc_d = nc.dram_tensor("bc", [KC, N_BASIS], bf16, kind="ExternalInput")
    n_otiles = K_SHARD // OBS_TILE  # 64
    x2l_d = nc.dram_tensor("x2l", [OBS_TILE, n_otiles], f32,
                           kind="ExternalInput")
    hb_d = nc.dram_tensor("hb", [OBS_TILE, 1], f32, kind="ExternalInput")
    out_d = nc.dram_tensor("out", [OBS_TILE, n_otiles], f32,
                           kind="ExternalOutput")

    blocks, n_pos_cols = _blocks(n_pos)
    n_cols = len(blocks)
    X_CHUNK = 2048
    C_CHUNK = 2048

    with tile.TileContext(nc) as tc:
        with (
            tc.tile_pool(name="const", bufs=1) as const,
            tc.tile_pool(name="amch", bufs=K_SHARD // X_CHUNK) as ampool,
            tc.tile_pool(name="acch", bufs=K_SHARD // X_CHUNK) as acpool,
            tc.tile_pool(name="bmch", bufs=N_BASIS // C_CHUNK) as bmpool,
            tc.tile_pool(name="bcch", bufs=N_BASIS // C_CHUNK) as bcpool,
            tc.tile_pool(name="acc", bufs=3) as apool,
            tc.tile_pool(name="red", bufs=2) as rpool,
            tc.tile_pool(name="gps", bufs=PSUM_BUFS, space="PSUM") as gpool,
        ):
            hb = const.tile([OBS_TILE, 1], f32)
            nc.sync.dma_start(out=hb[:], in_=hb_d.ap())
            x2l = const.tile([OBS_TILE, n_otiles], f32)
            nc.sync.dma_start(out=x2l[:], in_=x2l_d.ap())

            # only SP/ACT issue DMAs (HWDGE); gpsimd SWDGE pays DRAIN costs
            dma_engines = [nc.sync, nc.scalar]
            dma_rr = [0]

            def dma(out, in_):
                eng = dma_engines[dma_rr[0] % len(dma_engines)]
                dma_rr[0] += 1
                eng.dma_start(out=out, in_=in_)

            # obs chunk tiles (allocated up-front, loaded in priority order)
            amtiles = [ampool.tile([KM, X_CHUNK], bf16, name=f"amt{j}",
                                   tag="amt")
                       for j in range(K_SHARD // X_CHUNK)]
            actiles = [acpool.tile([KC, X_CHUNK], bf16, name=f"act{j}",
                                   tag="act")
                       for j in range(K_SHARD // X_CHUNK)]
            # bm/bc as single tiles: matmul moving slices cut at arbitrary
            # (n_pos-dependent) offsets and must not cross SBUF tile seams
            bm = bmpool.tile([KM, N_BASIS], bf16)
            bc = bcpool.tile([KC, N_BASIS], bf16)

            def xsl(j):
                return slice(j * X_CHUNK, (j + 1) * X_CHUNK)

            # priority: first obs chunk, then the basis tensors (finely
            # chunked across both queues), then the remaining obs chunks
            dma(amtiles[0][:], am_d.ap()[:, xsl(0)])
            dma(actiles[0][:], ac_d.ap()[:, xsl(0)])
            DC = 1024
            for j in range(N_BASIS // DC):
                sl = slice(j * DC, (j + 1) * DC)
                dma(bm[:, sl], bm_d.ap()[:, sl])
                dma(bc[:, sl], bc_d.ap()[:, sl])
            for j in range(1, K_SHARD // X_CHUNK):
                dma(amtiles[j][:], am_d.ap()[:, xsl(j)])
                dma(actiles[j][:], ac_d.ap()[:, xsl(j)])

            zbuf = const.tile([OBS_TILE, n_otiles], f32)

            for o in range(n_otiles):
                xj, xo = o * OBS_TILE // X_CHUNK, (o * OBS_TILE) % X_CHUNK
                lhs_m = amtiles[xj][:, xo:xo + OBS_TILE]
                lhs_c = actiles[xj][:, xo:xo + OBS_TILE]

                acc = apool.tile([OBS_TILE, n_cols], f32)
                for lo, hi, cidx in blocks:
                    sz = hi - lo
                    g = gpool.tile([OBS_TILE, BLOCK], f32, tag="gps")
                    # PSUM-bank-aligned sub-slices relative to block start
                    cuts = [(c, min(sz, c + BAS_SLICE))
                            for c in range(0, sz, BAS_SLICE)]
                    for lhs, bsrc, start, stop in (
                        (lhs_m, bm, True, False),
                        (lhs_c, bc, False, True),
                    ):
                        for s0, s1 in cuts:
                            nc.tensor.matmul(
                                g[:, s0:s1],
                                lhsT=lhs,
                                rhs=bsrc[:, lo + s0:lo + s1],
                                start=start, stop=stop,
                            )
                    nc.scalar.activation(
                        g[:, 0:sz], g[:, 0:sz],
                        mybir.ActivationFunctionType.Exp,
                        bias=x2l[:, o:o + 1],
                        accum_out=acc[:, cidx:cidx + 1],
                    )

                if n_pos_cols == 0:
                    nc.vector.reduce_sum(
                        zbuf[:, o:o + 1], acc[:], axis=mybir.AxisListType.X,
                        negate=True,
                    )
                elif n_pos_cols == n_cols:
                    nc.vector.reduce_sum(
                        zbuf[:, o:o + 1], acc[:], axis=mybir.AxisListType.X,
                    )
                else:
                    zp = rpool.tile([OBS_TILE, 1], f32)
                    zn = rpool.tile([OBS_TILE, 1], f32)
                    nc.vector.reduce_sum(
                        zp[:], acc[:, 0:n_pos_cols], axis=mybir.AxisListType.X,
                    )
                    nc.vector.reduce_sum(
                        zn[:], acc[:, n_pos_cols:n_cols],
                        axis=mybir.AxisListType.X,
                    )
                    nc.vector.tensor_sub(zbuf[:, o:o + 1], zp[:], zn[:])

            # sigmoid(z + b) = 0.5 + 0.5*tanh(0.5*z + 0.5*b), batched
            th = const.tile([OBS_TILE, n_otiles], f32)
            nc.scalar.activation(
                th[:], zbuf[:], mybir.ActivationFunctionType.Tanh,
                bias=hb[:], scale=0.5,
            )
            osb = const.tile([OBS_TILE, n_otiles], f32)
            nc.vector.tensor_scalar(
                out=osb[:], in0=th[:], scalar1=0.5, scalar2=0.5,
                op0=mybir.AluOpType.mult, op1=mybir.AluOpType.add,
            )
            nc.sync.dma_start(out=out_d.ap(), in_=osb[:])

    nc.compile()
    return nc


def _get_program(n_pos):
    if n_pos not in _prog_cache:
        _prog_cache[n_pos] = _build_program(n_pos)
    return _prog_cache[n_pos]


def _bf16(a):
    import ml_dtypes

    return a.astype(ml_dtypes.bfloat16)


def _prep_inputs(x, x_basis, w, b):
    x = np.asarray(x, dtype=np.float32)
    x_basis = np.asarray(x_basis, dtype=np.float32)
    w = np.asarray(w, dtype=np.float32).reshape(-1)
    b = np.asarray(b, dtype=np.float32).reshape(-1)

    pos = w >= 0
    perm = np.concatenate([np.nonzero(pos)[0], np.nonzero(~pos)[0]])
    n_pos = int(pos.sum())

    cb2 = 2.0 * x_basis[perm].T  # [64, N]
    lw = np.log(np.maximum(np.abs(w[perm]), 1e-35)).astype(np.float32)
    v = lw - np.sum(x_basis[perm] * x_basis[perm], axis=1)
    v_h = _bf16(v)
    v_l = _bf16(v - v_h.astype(np.float32))
    bh = _bf16(cb2)
    bl = _bf16(cb2 - bh.astype(np.float32))

    import ml_dtypes

    bm = np.empty((KM, N_BASIS), dtype=ml_dtypes.bfloat16)
    bm[:M_FEAT] = bh
    bm[M_FEAT] = _bf16(np.full(N_BASIS, -1.0, np.float32))
    bm[M_FEAT + 1] = v_h
    bm[M_FEAT + 2] = v_l
    bc = np.ascontiguousarray(np.concatenate([bh, bl], axis=0))

    hb = np.full((OBS_TILE, 1), 0.5 * b[0], dtype=np.float32)
    n_otiles = K_SHARD // OBS_TILE

    in_maps = []
    for c in range(N_CORES):
        xs = x[c * K_SHARD:(c + 1) * K_SHARD]
        xt = xs.T  # [64, 8192]
        x2 = np.sum(xs * xs, axis=1)
        x2h = _bf16(x2)
        x2l = -(x2 - x2h.astype(np.float32))  # fp32 remainder, negated
        ah = _bf16(xt)
        al = _bf16(xt - ah.astype(np.float32))

        am = np.empty((KM, K_SHARD), dtype=ml_dtypes.bfloat16)
        am[:M_FEAT] = ah
        am[M_FEAT] = x2h
        am[M_FEAT + 1] = _bf16(np.ones(K_SHARD, np.float32))
        am[M_FEAT + 2] = am[M_FEAT + 1]
        ac = np.ascontiguousarray(np.concatenate([al, ah], axis=0))

        in_maps.append({
            "am": np.ascontiguousarray(am),
            "ac": ac,
            "bm": np.ascontiguousarray(bm),
            "bc": bc,
            "x2l": np.ascontiguousarray(
                x2l.reshape(n_otiles, OBS_TILE).T.astype(np.float32)
            ),
            "hb": hb,
        })
    return in_maps, n_pos


LAST_EXEC_NS = None


def kernel(x, x_basis, w, b):
    global LAST_EXEC_NS
    import os

    from concourse.bass_utils import run_bass_kernel_spmd

    in_maps, n_pos = _prep_inputs(x, x_basis, w, b)
    nc = _get_program(n_pos)

    trace = bool(os.environ.get("RBF_TRACE"))
    kwargs = {}
    if trace:
        tmpdir = os.environ.get("RBF_TRACE_DIR") or None
        kwargs = {"trace": True, "tmpdir": tmpdir}
    res = run_bass_kernel_spmd(nc, in_maps, list(range(N_CORES)), **kwargs)
    LAST_EXEC_NS = res.exec_time_ns

    # zout[p, o] = out for obs o*128 + p
    out = np.concatenate(
        [res.results[c]["out"].T.reshape(K_SHARD) for c in range(N_CORES)]
    )
    return out.reshape(K_FULL, 1).astype(np.float32)


# revision 21
# speedup vs baseline: 3.1089x; 1.1707x over previous
"""Trainium2 Bass kernel for nn_LogisticRegressionRBF.

reference:
    sq[i,j] = ||x_i||^2 + ||c_j||^2 - 2 x_i.c_j     (K=65536 obs, N=4096 basis)
    out     = sigmoid(exp(-sq) @ w.T + b)           [K, 1]

Strategy (data-parallel over obs across 8 cores, 8192 obs/core):
  * Basis centers are permuted so the ones with w_j >= 0 come first
    (n_pos of them), and ln|w_j| is folded into the matmul, so exp directly
    produces |w_j| * exp(-sq); the free-dim sum of that IS the weighted
    reduction, which the ACT engine computes for free via accum_out.
  * fp32 PE matmuls are slow on TRN2 (LOW_HIGH double pass + serial
    non-FWL weight loads), so the distance matmul runs as a compensated
    bf16 pair accumulating into the same PSUM slice:
      main [67 rows]:  Ah(x) . Bh(2c)  - bf16(x2) + v_h + v_l
                        (v = ln|w| - ||c||^2 split into two bf16 rows)
      corr [128 rows]: Al(x) . Bh(2c)  +  Ah(x) . Bl(2c)
    and the fp32 remainder of ||x||^2 is applied exactly as the ACT
    per-partition bias:  exp(G + (-x2_lo)).  Total argument error ~4e-4.
  * ACT exp runs in-place on [128, 2048] PSUM ping-pong blocks with
    accum_out -> sign-pure accumulator columns; DVE combines:
    z = sum(pos cols) - sum(neg cols).
  * sigmoid via tanh (same ACT table set as exp -> no table reload):
       sigmoid(z + b) = 0.5 + 0.5 * tanh(0.5*z + 0.5*b), batched [128, 64].
Output layout: zout[p, o] = out for obs o*128 + p; host transposes.
"""

import sys

if "/opt/trn_rl_repo" not in sys.path:
    sys.path.insert(0, "/opt/trn_rl_repo")

import numpy as np

K_FULL, N_BASIS, M_FEAT = 65536, 4096, 64
N_CORES = 8
K_SHARD = K_FULL // N_CORES  # 8192
KM = M_FEAT + 4  # 68 main rows (incl. bf16 remainder of ||x||^2)
KC = 2 * M_FEAT  # 128 correction rows
OBS_TILE = 128  # obs per stationary tile / psum partitions
BAS_SLICE = 512  # matmul moving free dim (one PSUM bank of fp32)
BLOCK = 1024  # basis cols per ACT block (2 PSUM banks)
PSUM_BUFS = 4

_prog_cache: dict = {}


def _blocks(n_pos):
    """Sign-pure PSUM blocks (lo, hi, col): the basis axis is cut exactly at
    n_pos, each side chunked into <=BLOCK pieces.  Emitted smallest-first so
    tiny remainder blocks lead the pipeline instead of straggling.
    Accumulator columns: positives get [0, n_pos_cols)."""
    blocks = []
    for lo, hi, pos in [(0, n_pos, True), (n_pos, N_BASIS, False)]:
        cur = lo
        while cur < hi:
            sz = min(BLOCK, hi - cur)
            # leave the remainder as its own block
            if (hi - cur) % BLOCK and hi - cur > BLOCK:
                sz = (hi - cur) % BLOCK
            blocks.append([cur, cur + sz, pos])
            cur += sz
    blocks.sort(key=lambda b: b[1] - b[0])
    n_pos_cols = sum(1 for b in blocks if b[2])
    pc, nc_ = 0, n_pos_cols
    out = []
    for lo, hi, pos in blocks:
        if pos:
            out.append((lo, hi, pc))
            pc += 1
        else:
            out.append((lo, hi, nc_))
            nc_ += 1

    # balance the per-block reduction between ACT accum_out (+287ns each)
    # and DVE tensor_reduce (sz*1.042+250 ns each, engine otherwise idle)
    act_ns = N_BASIS * 0.833 + len(out) * 143.0
    dve_ns = 500.0
    use_dve = {}
    for lo, hi, col in sorted(out, key=lambda b: b[0] - b[1]):
        cost = (hi - lo) * 1.042 + 250.0
        if dve_ns + cost <= act_ns + 287.0:
            use_dve[col] = True
            dve_ns += cost
        else:
            use_dve[col] = False
            act_ns += 287.0
    return out, n_pos_cols, use_dve


def _build_program(n_pos):
    import concourse.bacc as bacc
    import concourse.mybir as mybir
    import concourse.tile as tile

    f32 = mybir.dt.float32
    bf16 = mybir.dt.bfloat16
    nc = bacc.Bacc("TRN2", target_bir_lowering=False, debug=False,
                   num_devices=N_CORES)

    am_d = nc.dram_tensor("am", [KM, K_SHARD], bf16, kind="ExternalInput")
    ac_d = nc.dram_tensor("ac", [KC, K_SHARD], bf16, kind="ExternalInput")
    bm_d = nc.dram_tensor("bm", [KM, N_BASIS], bf16, kind="ExternalInput")
    bc_d = nc.dram_tensor("bc", [KC, N_BASIS], bf16, kind="ExternalInput")
    n_otiles = K_SHARD // OBS_TILE  # 64
    hb_d = nc.dram_tensor("hb", [OBS_TILE, 1], f32, kind="ExternalInput")
    out_d = nc.dram_tensor("out", [OBS_TILE, n_otiles], f32,
                           kind="ExternalOutput")

    blocks, n_pos_cols, use_dve = _blocks(n_pos)
    n_cols = len(blocks)
    X_CHUNK = 2048
    C_CHUNK = 2048

    with tile.TileContext(nc) as tc:
        with (
            tc.tile_pool(name="const", bufs=1) as const,
            tc.tile_pool(name="amch", bufs=K_SHARD // X_CHUNK) as ampool,
            tc.tile_pool(name="acch", bufs=K_SHARD // X_CHUNK) as acpool,
            tc.tile_pool(name="bmch", bufs=N_BASIS // C_CHUNK) as bmpool,
            tc.tile_pool(name="bcch", bufs=N_BASIS // C_CHUNK) as bcpool,
            tc.tile_pool(name="acc", bufs=3) as apool,
            tc.tile_pool(name="red", bufs=2) as rpool,
            tc.tile_pool(name="gps", bufs=PSUM_BUFS, space="PSUM") as gpool,
        ):
            # obs chunk tiles (allocated up-front, loaded in priority order)
            amtiles = [ampool.tile([KM, X_CHUNK], bf16, name=f"amt{j}",
                                   tag="amt")
                       for j in range(K_SHARD // X_CHUNK)]
            actiles = [acpool.tile([KC, X_CHUNK], bf16, name=f"act{j}",
                                   tag="act")
                       for j in range(K_SHARD // X_CHUNK)]
            # bm/bc as single tiles: matmul moving slices cut at arbitrary
            # (n_pos-dependent) offsets and must not cross SBUF tile seams
            bm = bmpool.tile([KM, N_BASIS], bf16)
            bc = bcpool.tile([KC, N_BASIS], bf16)
            hb = const.tile([OBS_TILE, 1], f32)

            def xsl(j):
                return slice(j * X_CHUNK, (j + 1) * X_CHUNK)

            # two HWDGE queues (SP, ACT); per queue, gating tensors first:
            # main path (am0+bm) on SP, correction path (ac0+bc) on ACT
            nc.sync.dma_start(out=amtiles[0][:], in_=am_d.ap()[:, xsl(0)])
            nc.scalar.dma_start(out=actiles[0][:], in_=ac_d.ap()[:, xsl(0)])
            nc.sync.dma_start(out=bm[:], in_=bm_d.ap())
            nc.scalar.dma_start(out=bc[:], in_=bc_d.ap())
            nc.sync.dma_start(out=hb[:], in_=hb_d.ap())
            for j in range(1, K_SHARD // X_CHUNK):
                eng = nc.sync if j % 2 else nc.scalar
                eng.dma_start(out=amtiles[j][:], in_=am_d.ap()[:, xsl(j)])
                eng2 = nc.scalar if j % 2 else nc.sync
                eng2.dma_start(out=actiles[j][:], in_=ac_d.ap()[:, xsl(j)])

            zbuf = const.tile([OBS_TILE, n_otiles], f32)

            for o in range(n_otiles):
                xj, xo = o * OBS_TILE // X_CHUNK, (o * OBS_TILE) % X_CHUNK
                lhs_m = amtiles[xj][:, xo:xo + OBS_TILE]
                lhs_c = actiles[xj][:, xo:xo + OBS_TILE]

                acc = apool.tile([OBS_TILE, n_cols], f32)
                for lo, hi, cidx in blocks:
                    sz = hi - lo
                    g = gpool.tile([OBS_TILE, BLOCK], f32, tag="gps")
                    # PSUM-bank-aligned sub-slices relative to block start
                    cuts = [(c, min(sz, c + BAS_SLICE))
                            for c in range(0, sz, BAS_SLICE)]
                    for lhs, bsrc, start, stop in (
                        (lhs_m, bm, True, False),
                        (lhs_c, bc, False, True),
                    ):
                        for s0, s1 in cuts:
                            nc.tensor.matmul(
                                g[:, s0:s1],
                                lhsT=lhs,
                                rhs=bsrc[:, lo + s0:lo + s1],
                                start=start, stop=stop,
                            )
                    if use_dve[cidx]:
                        nc.scalar.activation(
                            g[:, 0:sz], g[:, 0:sz],
                            mybir.ActivationFunctionType.Exp,
                        )
                        nc.vector.reduce_sum(
                            acc[:, cidx:cidx + 1], g[:, 0:sz],
                            axis=mybir.AxisListType.X,
                        )
                    else:
                        nc.scalar.activation(
                            g[:, 0:sz], g[:, 0:sz],
                            mybir.ActivationFunctionType.Exp,
                            accum_out=acc[:, cidx:cidx + 1],
                        )

                if n_pos_cols == 0:
                    nc.vector.reduce_sum(
                        zbuf[:, o:o + 1], acc[:], axis=mybir.AxisListType.X,
                        negate=True,
                    )
                elif n_pos_cols == n_cols:
                    nc.vector.reduce_sum(
                        zbuf[:, o:o + 1], acc[:], axis=mybir.AxisListType.X,
                    )
                else:
                    zp = rpool.tile([OBS_TILE, 1], f32)
                    zn = rpool.tile([OBS_TILE, 1], f32)
                    nc.vector.reduce_sum(
                        zp[:], acc[:, 0:n_pos_cols], axis=mybir.AxisListType.X,
                    )
                    nc.vector.reduce_sum(
                        zn[:], acc[:, n_pos_cols:n_cols],
                        axis=mybir.AxisListType.X,
                    )
                    nc.vector.tensor_sub(zbuf[:, o:o + 1], zp[:], zn[:])

            # sigmoid(z + b) = 0.5 + 0.5*tanh(0.5*z + 0.5*b), batched
            th = const.tile([OBS_TILE, n_otiles], f32)
            nc.scalar.activation(
                th[:], zbuf[:], mybir.ActivationFunctionType.Tanh,
                bias=hb[:], scale=0.5,
            )
            osb = const.tile([OBS_TILE, n_otiles], f32)
            nc.vector.tensor_scalar(
                out=osb[:], in0=th[:], scalar1=0.5, scalar2=0.5,
                op0=mybir.AluOpType.mult, op1=mybir.AluOpType.add,
            )
            nc.sync.dma_start(out=out_d.ap(), in_=osb[:])

    nc.compile()
    return nc


def _get_program(n_pos):
    if n_pos not in _prog_cache:
        _prog_cache[n_pos] = _build_program(n_pos)
    return _prog_cache[n_pos]


def _bf16(a):
    import ml_dtypes

    return a.astype(ml_dtypes.bfloat16)


def _prep_inputs(x, x_basis, w, b):
    x = np.asarray(x, dtype=np.float32)
    x_basis = np.asarray(x_basis, dtype=np.float32)
    w = np.asarray(w, dtype=np.float32).reshape(-1)
    b = np.asarray(b, dtype=np.float32).reshape(-1)

    pos = w >= 0
    perm = np.concatenate([np.nonzero(pos)[0], np.nonzero(~pos)[0]])
    n_pos = int(pos.sum())

    cb2 = 2.0 * x_basis[perm].T  # [64, N]
    lw = np.log(np.maximum(np.abs(w[perm]), 1e-35)).astype(np.float32)
    v = lw - np.sum(x_basis[perm] * x_basis[perm], axis=1)
    v_h = _bf16(v)
    v_l = _bf16(v - v_h.astype(np.float32))
    bh = _bf16(cb2)
    bl = _bf16(cb2 - bh.astype(np.float32))

    import ml_dtypes

    bm = np.empty((KM, N_BASIS), dtype=ml_dtypes.bfloat16)
    bm[:M_FEAT] = bh
    bm[M_FEAT] = _bf16(np.full(N_BASIS, -1.0, np.float32))
    bm[M_FEAT + 1] = v_h
    bm[M_FEAT + 2] = v_l
    bm[M_FEAT + 3] = bm[M_FEAT]  # -1 against the ||x||^2 bf16 remainder
    bc = np.ascontiguousarray(np.concatenate([bh, bl], axis=0))

    hb = np.full((OBS_TILE, 1), 0.5 * b[0], dtype=np.float32)
    n_otiles = K_SHARD // OBS_TILE

    in_maps = []
    for c in range(N_CORES):
        xs = x[c * K_SHARD:(c + 1) * K_SHARD]
        xt = xs.T  # [64, 8192]
        x2 = np.sum(xs * xs, axis=1)
        x2h = _bf16(x2)
        x2l = x2 - x2h.astype(np.float32)  # fp32 remainder
        ah = _bf16(xt)
        al = _bf16(xt - ah.astype(np.float32))

        am = np.empty((KM, K_SHARD), dtype=ml_dtypes.bfloat16)
        am[:M_FEAT] = ah
        am[M_FEAT] = x2h
        am[M_FEAT + 1] = _bf16(np.ones(K_SHARD, np.float32))
        am[M_FEAT + 2] = am[M_FEAT + 1]
        am[M_FEAT + 3] = _bf16(x2l)
        ac = np.ascontiguousarray(np.concatenate([al, ah], axis=0))

        in_maps.append({
            "am": np.ascontiguousarray(am),
            "ac": ac,
            "bm": np.ascontiguousarray(bm),
            "bc": bc,
            "hb": hb,
        })
    return in_maps, n_pos


LAST_EXEC_NS = None


def kernel(x, x_basis, w, b):
    global LAST_EXEC_NS
    import os

    from concourse.bass_utils import run_bass_kernel_spmd

    in_maps, n_pos = _prep_inputs(x, x_basis, w, b)
    nc = _get_program(n_pos)

    trace = bool(os.environ.get("RBF_TRACE"))
    kwargs = {}
    if trace:
        tmpdir = os.environ.get("RBF_TRACE_DIR") or None
        kwargs = {"trace": True, "tmpdir": tmpdir}
    res = run_bass_kernel_spmd(nc, in_maps, list(range(N_CORES)), **kwargs)
    LAST_EXEC_NS = res.exec_time_ns

    # zout[p, o] = out for obs o*128 + p
    out = np.concatenate(
        [res.results[c]["out"].T.reshape(K_SHARD) for c in range(N_CORES)]
    )
    return out.reshape(K_FULL, 1).astype(np.float32)
